# revision 1
# baseline (speedup 1.0000x reference)
"""Self-contained Trainium2 Bass kernel for nn_DFINESegTransformer.

kernel(**inputs) -> [4, 300, 160, 160] float32 mask logits.

Sharding: 8 NeuronCores = 4 batches x 2 output-row halves. Every core runs
an identical Bass/Tile program; the host mirrors (flips H) the input strip
and the depthwise taps for odd halves and un-flips their output rows.

Per-core pipeline: 2x ConvNeXt block [dwconv3x3 as 9 diagonal PE matmuls ->
channel-LN via ones-matmul stats + folded affine -> 1x1 conv (ln_w folded)
-> exact Gelu (ACT table) -> residual] -> sproj 1x1 -> separable bilinear 2x
upsample (scale factors folded into query weights) -> per-query mask einsum.
Heavy operands in bf16 (PE streams 1 col/cycle vs 4 for fp32); LN stats,
alpha/beta math, gelu input and the output stay fp32.
"""
import numpy as np
import ml_dtypes

import jax
from jax.sharding import Mesh, PartitionSpec
try:
    from jax.experimental.shard_map import shard_map
except ImportError:
    from jax.shard_map import shard_map

import concourse.bacc as bacc
import concourse.bass as bass
import concourse.tile as tile
from concourse import mybir
from concourse.bass2jax import (_bass_exec_p, install_neuronx_cc_hook,
                                partition_id_tensor)
from contextlib import ExitStack

class CachedRunner:
    def __init__(self, nc, n_cores=8):
        install_neuronx_cc_hook()
        self.n_cores = n_cores
        partition_name = nc.partition_id_tensor.name if nc.partition_id_tensor else None
        in_names, out_names, out_avals, zero_shapes = [], [], [], []
        for alloc in nc.m.functions[0].allocations:
            if not isinstance(alloc, mybir.MemoryLocationSet):
                continue
            name = alloc.memorylocations[0].name
            if alloc.kind == "ExternalInput":
                if name != partition_name:
                    in_names.append(name)
            elif alloc.kind == "ExternalOutput":
                out_names.append(name)
                shape = tuple(alloc.tensor_shape)
                dtype = mybir.dt.np(alloc.dtype)
                out_avals.append(jax.core.ShapedArray(shape, dtype))
                zero_shapes.append((shape, dtype))
        self.n_params = len(in_names)
        self.in_names = list(in_names)
        self.out_names = out_names
        self.out_avals = out_avals
        self.zero_shapes = zero_shapes
        all_in = in_names + out_names
        if partition_name is not None:
            all_in.append(partition_name)

        donate = tuple(range(self.n_params, self.n_params + len(out_names)))

        def _body(*args):
            operands = list(args)
            if partition_name is not None:
                operands.append(partition_id_tensor())
            outs = _bass_exec_p.bind(
                *operands,
                out_avals=tuple(out_avals),
                in_names=tuple(all_in),
                out_names=tuple(out_names),
                lowering_input_output_aliases=(),
                sim_require_finite=True,
                sim_require_nnan=True,
                nc=nc,
            )
            return tuple(outs)

        devices = jax.devices()[:n_cores]
        self.mesh = Mesh(np.asarray(devices), ("core",))
        in_specs = (PartitionSpec("core"),) * (self.n_params + len(out_names))
        out_specs = (PartitionSpec("core"),) * len(out_names)
        self.fn = jax.jit(
            shard_map(_body, mesh=self.mesh, in_specs=in_specs,
                      out_specs=out_specs, check_rep=False),
            donate_argnums=donate, keep_unused=True)

    def concat_inputs(self, in_maps):
        return [np.concatenate([np.asarray(m[n]) for m in in_maps], axis=0)
                for n in self.in_names]

    def zeros(self):
        return [np.zeros((self.n_cores * s[0], *s[1:]), d)
                for (s, d) in self.zero_shapes]

    def run_device(self, concat_in, zeros):
        """Returns device arrays (not transferred)."""
        outs = self.fn(*concat_in, *zeros)
        return outs

    def __call__(self, in_maps):
        outs = self.run_device(self.concat_inputs(in_maps), self.zeros())
        res = []
        for c in range(self.n_cores):
            res.append({
                n: np.asarray(outs[i]).reshape(self.n_cores, *self.out_avals[i].shape)[c]
                for i, n in enumerate(self.out_names)})
        return res


F32 = mybir.dt.float32
BF = mybir.dt.bfloat16
AF = mybir.ActivationFunctionType
OP = mybir.AluOpType

C = 256
EPS = 1e-6
R = 43            # strip rows per core
PR, PW = 45, 82   # padded strip
MR = 41           # mask rows (sproj output rows)
SFR = 42          # sf rows incl. replicated top row
NQ = 300
OH, OW = 80, 160  # output rows/cols per core

DW_CHUNKS = [(i * 6, min(R, i * 6 + 6)) for i in range((R + 5) // 6)]
SP_CHUNKS = [(i * 6, min(MR, i * 6 + 6)) for i in range((MR + 5) // 6)]
UP_CHUNKS = [(i * 6, min(40, i * 6 + 6)) for i in range((40 + 5) // 6)]

TAPS = [(dy, dx) for dy in (-1, 0, 1) for dx in (-1, 0, 1)]
PE_TAPS = TAPS

N_TILES = [(0, 128), (128, 128), (256, 44)]
SROW = 3456  # padded stats row length (27*128)

BF16_INPUTS = {"xin", "dw_diag", "pw_lhsT", "sproj_lhsT", "qfT",
               "w1T", "w2T", "w3qT"}
INPUT_SPECS = [
    ("xin", (2, 128, PR, PW)),
    ("dw_diag", (2, 9, 2, 128, 128)),
    ("dwb", (2, 2, 128, 1)),
    ("pw_lhsT", (2, 2, 2, 128, 128)),
    ("pw_colwp", (2, 2, 128, 1)),
    ("pw_be", (2, 2, 128, 1)),
    ("sproj_lhsT", (2, 128, 128)),
    ("sproj_b", (128, 1)),
    ("qfT", (2, 128, NQ)),
    ("w1T", (2, 128, 512)),
    ("b1", (4, 128, 1)),
    ("w2T", (4, 128, 512)),
    ("b2", (4, 128, 1)),
    ("w3qT", (4, 128, 128)),
    ("b3q", (128, 1)),
    ("hb", (128, 1)),
]


def build_program():
    nc = bacc.Bacc("TRN2", target_bir_lowering=False, debug=False, num_devices=8)

    di = {}
    for name, shape in INPUT_SPECS:
        dt = BF if name in BF16_INPUTS else F32
        di[name] = nc.dram_tensor(name, list(shape), dt, kind="ExternalInput")
    y = nc.dram_tensor("y", [NQ, OH, OW], F32, kind="ExternalOutput")
    scratch = nc.dram_tensor("scratch", [2, SROW], BF)

    with tile.TileContext(nc) as tc, ExitStack() as ctx:
        consts = ctx.enter_context(tc.tile_pool(name="consts", bufs=1))
        states = ctx.enter_context(tc.tile_pool(name="states", bufs=2))
        zpool = ctx.enter_context(tc.tile_pool(name="zpool", bufs=2))
        sfpool = ctx.enter_context(tc.tile_pool(name="sfpool", bufs=1))
        small = ctx.enter_context(tc.tile_pool(name="small", bufs=10))
        upool = ctx.enter_context(tc.tile_pool(name="upool", bufs=2))
        stg = ctx.enter_context(tc.tile_pool(name="stg", bufs=4))
        qpool = ctx.enter_context(tc.tile_pool(name="qpool", bufs=1))
        dwpool = ctx.enter_context(tc.tile_pool(name="dwpool", bufs=1))
        rowp = ctx.enter_context(tc.tile_pool(name="rowp", bufs=1))
        bcp = ctx.enter_context(tc.tile_pool(name="bcp", bufs=1))

        pp_z = ctx.enter_context(tc.tile_pool(name="pp_z", bufs=2, space="PSUM"))
        pp_stat = ctx.enter_context(tc.tile_pool(name="pp_stat", bufs=2, space="PSUM"))
        pp_mm = ctx.enter_context(tc.tile_pool(name="pp_mm", bufs=2, space="PSUM"))

        # ---- weights/constants -> SBUF ----
        dwb = consts.tile([128, 2, 2], F32, tag="dwb")
        pw_lhsT = consts.tile([128, 2, 2, 2, 128], BF, tag="pw_lhsT")
        pw_colwp = consts.tile([128, 2, 2], F32, tag="pw_colwp")
        pw_be = consts.tile([128, 2, 2], F32, tag="pw_be")
        sproj_lhsT = consts.tile([128, 2, 128], BF, tag="sproj_lhsT")
        sproj_b = consts.tile([128, 1], F32, tag="sproj_b")
        qfT = consts.tile([128, 2, NQ], BF, tag="qfT")
        w1T = consts.tile([128, 2, 512], BF, tag="w1T")
        b1 = consts.tile([128, 4], F32, tag="b1")
        w2T = consts.tile([128, 4, 512], BF, tag="w2T")
        b2 = consts.tile([128, 4], F32, tag="b2")
        w3qT = consts.tile([128, 4, 128], BF, tag="w3qT")
        b3q = consts.tile([128, 1], F32, tag="b3q")
        hb = consts.tile([128, 1], F32, tag="hb")
        ones_col = consts.tile([128, 1], BF, tag="ones_col")
        eps_t = consts.tile([128, 1], F32, tag="eps_t")
        ones_row = consts.tile([1, 128], F32, tag="ones_row")

        A = lambda n: di[n].ap()
        nc.sync.dma_start(out=dwb[:], in_=A("dwb").rearrange("b c k o -> k b (c o)"))
        nc.sync.dma_start(out=pw_lhsT[:], in_=A("pw_lhsT").rearrange("b t m k o -> k b t m o"))
        nc.sync.dma_start(out=pw_colwp[:], in_=A("pw_colwp").rearrange("b m k o -> k b (m o)"))
        nc.sync.dma_start(out=pw_be[:], in_=A("pw_be").rearrange("b m k o -> k b (m o)"))
        nc.sync.dma_start(out=sproj_lhsT[:], in_=A("sproj_lhsT").rearrange("t k m -> k t m"))
        nc.sync.dma_start(out=sproj_b[:], in_=A("sproj_b"))
        nc.sync.dma_start(out=qfT[:], in_=A("qfT").rearrange("t k m -> k t m"))
        nc.sync.dma_start(out=w1T[:], in_=A("w1T").rearrange("t k m -> k t m"))
        nc.sync.dma_start(out=b1[:], in_=A("b1").rearrange("m k o -> k (m o)"))
        nc.sync.dma_start(out=w2T[:], in_=A("w2T").rearrange("t k m -> k t m"))
        nc.sync.dma_start(out=b2[:], in_=A("b2").rearrange("m k o -> k (m o)"))
        nc.sync.dma_start(out=w3qT[:], in_=A("w3qT").rearrange("t k m -> k t m"))
        nc.sync.dma_start(out=b3q[:], in_=A("b3q"))
        nc.sync.dma_start(out=hb[:], in_=A("hb"))
        nc.vector.memset(ones_col[:], 1.0)
        nc.vector.memset(eps_t[:], EPS)

        nc.vector.memset(ones_row[:], 1.0)

        x_state = states.tile([128, 2, PR, PW], BF, tag="state")
        nc.sync.dma_start(out=x_state[:], in_=A("xin").rearrange("c k h w -> k c h w"))

        # ---- query MLP ----
        q1 = qpool.tile([128, 4, NQ], BF, tag="q1")
        for mt in range(4):
            ps = pp_mm.tile([128, NQ], F32, tag="mm")
            for kt in range(2):
                nc.tensor.matmul(ps[:], w1T[:, kt, mt * 128:(mt + 1) * 128],
                                 qfT[:, kt, :], start=(kt == 0), stop=(kt == 1))
            nc.scalar.activation(q1[:, mt, :], ps[:], AF.Relu, bias=b1[:, mt:mt + 1])
        q2 = qpool.tile([128, 4, NQ], BF, tag="q2")
        for mt in range(4):
            ps = pp_mm.tile([128, NQ], F32, tag="mm")
            for kt in range(4):
                nc.tensor.matmul(ps[:], w2T[:, kt, mt * 128:(mt + 1) * 128],
                                 q1[:, kt, :], start=(kt == 0), stop=(kt == 3))
            nc.scalar.activation(q2[:, mt, :], ps[:], AF.Relu, bias=b2[:, mt:mt + 1])
        qT = qpool.tile([128, NQ], BF, tag="qT")
        ps = pp_mm.tile([128, NQ], F32, tag="mm")
        for kt in range(4):
            nc.tensor.matmul(ps[:], w3qT[:, kt, :], q2[:, kt, :],
                             start=(kt == 0), stop=(kt == 3))
        nc.scalar.activation(qT[:], ps[:], AF.Identity, bias=b3q[:])

        # ---- conv blocks ----
        def conv_block(blk, xst, pad_out):
            dw_diag = dwpool.tile([128, 9, 2, 128], BF, tag="dw_diag")
            nc.gpsimd.dma_start(out=dw_diag[:],
                                in_=A("dw_diag")[blk].rearrange("t c k m -> k t c m"))
            z = zpool.tile([128, 2, R, 80], BF, tag="z")
            um_rows = rowp.tile([33, SROW], F32, tag="um_rows")
            nc.vector.memset(um_rows[0:1, 3440:], 0.0)
            nc.vector.memset(um_rows[32:33, 3440:], 256.0)

            # pass A: dw conv + stats; tap-outer over 4-chunk superchunks
            for sc in range(4):
                chunks = DW_CHUNKS[sc * 2:(sc + 1) * 2]
                for ct in range(2):
                    zpss = []
                    for ci, (r0, r1) in enumerate(chunks):
                        zps_t = pp_z.tile([128, r1 - r0, 80], F32, tag="zps")
                        zpss.append(zps_t)
                    for i, (dy, dx) in enumerate(PE_TAPS):
                        for ci, (r0, r1) in enumerate(chunks):
                            nc.tensor.matmul(
                                zpss[ci][:], dw_diag[:, i, ct, :],
                                xst[:, ct, 1 + r0 + dy:1 + r1 + dy, 1 + dx:81 + dx],
                                start=(i == 0), stop=(i == len(PE_TAPS) - 1))
                    for ci, (r0, r1) in enumerate(chunks):
                        zc = z[:, ct, r0:r1, :]
                        nc.vector.tensor_scalar(
                            zc, zpss[ci][:], 1.0, dwb[:, blk, ct:ct + 1],
                            OP.mult, OP.add)
                # stats for this superchunk (both ctiles of z ready)
                for (r0, r1) in chunks:
                    nr = r1 - r0
                    stps = pp_stat.tile([33, nr * 80], F32, tag="stat")
                    for ct in range(2):
                        zc = z[:, ct, r0:r1, :]
                        zsq = small.tile([128, nr, 80], BF, tag="tmp")
                        nc.scalar.activation(zsq[:], zc, AF.Square)
                        nc.tensor.matmul(stps[0:1, :], ones_col[:], zc,
                                         start=(ct == 0), stop=(ct == 1),
                                         skip_group_check=True)
                        nc.tensor.matmul(stps[32:33, :], ones_col[:], zsq[:],
                                         start=(ct == 0), stop=(ct == 1),
                                         skip_group_check=True)
                    nc.scalar.copy(um_rows[0:1, r0 * 80:r1 * 80], stps[0:1, :])
                    nc.scalar.copy(um_rows[32:33, r0 * 80:r1 * 80], stps[32:33, :])

            # alpha/beta on packed [128, 27]
            u_pk = small.tile([128, 27], F32, tag="pk")
            m2_pk = small.tile([128, 27], F32, tag="pk")
            nc.sync.dma_start(out=u_pk[:], in_=um_rows[0:1, :])
            nc.sync.dma_start(out=m2_pk[:], in_=um_rows[32:33, :])
            t_pk = small.tile([128, 27], F32, tag="pk")
            nc.vector.tensor_scalar_mul(t_pk[:], u_pk[:], 1.0 / C)
            sq = small.tile([128, 27], F32, tag="pk")
            nc.scalar.activation(sq[:], t_pk[:], AF.Square)
            d_pk = small.tile([128, 27], F32, tag="pk")
            nc.vector.scalar_tensor_tensor(d_pk[:], m2_pk[:], 1.0 / C, sq[:],
                                           OP.mult, OP.subtract)
            sd = small.tile([128, 27], F32, tag="pk")
            nc.scalar.activation(sd[:], d_pk[:], AF.Sqrt, bias=eps_t[:])
            a_pk = small.tile([128, 27], BF, tag="pk")
            b_pk = small.tile([128, 27], BF, tag="pk")
            with nc.allow_low_precision(reason="alpha/beta rows cast to bf16"):
                nc.vector.reciprocal(a_pk[:], sd[:])
                nc.vector.tensor_scalar_mul(b_pk[:], t_pk[:], -1.0)
            nc.sync.dma_start(out=scratch.ap()[0:1, :], in_=a_pk[:])
            nc.sync.dma_start(out=scratch.ap()[1:2, :], in_=b_pk[:])
            a_bc = bcp.tile([128, SROW], BF, tag="a_bc")
            b_bc = bcp.tile([128, SROW], BF, tag="b_bc")
            HALF = SROW // 2
            for hh, eng in ((0, nc.scalar), (1, nc.sync)):
                src_a = bass.AP(tensor=scratch, offset=hh * HALF,
                                ap=[[0, 128], [1, HALF]])
                src_b = bass.AP(tensor=scratch, offset=SROW + hh * HALF,
                                ap=[[0, 128], [1, HALF]])
                eng.dma_start(out=a_bc[:, hh * HALF:(hh + 1) * HALF], in_=src_a)
                eng.dma_start(out=b_bc[:, hh * HALF:(hh + 1) * HALF], in_=src_b)

            # pass B: pw + gelu + resid
            out = states.tile([128, 2, PR, PW], BF, tag="state")
            if pad_out:
                nc.gpsimd.memset(out[:, :, 0, :], 0.0)
                nc.gpsimd.memset(out[:, :, PR - 1, :], 0.0)
                nc.gpsimd.memset(out[:, :, :, 0], 0.0)
                nc.gpsimd.memset(out[:, :, :, PW - 1], 0.0)
            for (r0, r1) in DW_CHUNKS:
                nr = r1 - r0
                for mt in range(2):
                    pwps = pp_mm.tile([128, nr, 80], F32, tag="mm")
                    for kt in range(2):
                        nc.tensor.matmul(pwps[:], pw_lhsT[:, blk, kt, mt, :],
                                         z[:, kt, r0:r1, :],
                                         start=(kt == 0), stop=(kt == 1))
                    t1 = small.tile([128, nr, 80], F32, tag="tmp")
                    nc.vector.scalar_tensor_tensor(
                        t1[:], b_bc[:, r0 * 80:r1 * 80].rearrange("k (r w) -> k r w", w=80),
                        pw_colwp[:, blk, mt:mt + 1], pwps[:], OP.mult, OP.add)
                    g = small.tile([128, nr, 80], F32, tag="tmp")
                    geng = nc.vector if mt == 0 else nc.gpsimd
                    geng.tensor_tensor(
                        g[:], t1[:],
                        a_bc[:, r0 * 80:r1 * 80].rearrange("k (r w) -> k r w", w=80),
                        OP.mult)
                    gel = small.tile([128, nr, 80], BF, tag="tmp")
                    nc.scalar.activation(gel[:], g[:], AF.Gelu,
                                         bias=pw_be[:, blk, mt:mt + 1])
                    eng = nc.vector if mt == 0 else nc.gpsimd
                    eng.tensor_tensor(
                        out[:, mt, 1 + r0:1 + r1, 1:81], gel[:],
                        xst[:, mt, 1 + r0:1 + r1, 1:81], OP.add)
            return out

        b1st = conv_block(0, x_state, True)
        b2st = conv_block(1, b1st, False)

        # ---- sproj -> sf ----
        sf = sfpool.tile([128, SFR, PW], BF, tag="sf")
        for (r0, r1) in SP_CHUNKS:
            nr = r1 - r0
            sps = pp_mm.tile([128, nr, 80], F32, tag="mm")
            for kt in range(2):
                nc.tensor.matmul(sps[:], sproj_lhsT[:, kt, :],
                                 b2st[:, kt, 1 + r0:1 + r1, 1:81],
                                 start=(kt == 0), stop=(kt == 1))
            nc.vector.tensor_scalar_add(sf[:, 1 + r0:1 + r1, 1:81], sps[:], sproj_b[:])
        nc.vector.tensor_copy(sf[:, 0, 1:81], sf[:, 1, 1:81])
        nc.vector.tensor_copy(sf[:, :, 0], sf[:, :, 1])
        nc.vector.tensor_copy(sf[:, :, 81], sf[:, :, 80])

        # ---- upsample + einsum + out ----
        for (p0, p1) in UP_CHUNKS:
            npair = p1 - p0
            nsf = npair + 2
            THIRD = 1.0 / 3.0
            wt = upool.tile([128, 8, OW], BF, tag="wt")
            nc.vector.scalar_tensor_tensor(
                wt[:, :nsf, 0:OW:2], sf[:, p0:p0 + nsf, 0:80], THIRD,
                sf[:, p0:p0 + nsf, 1:81], OP.mult, OP.add)
            nc.vector.scalar_tensor_tensor(
                wt[:, :nsf, 1:OW:2], sf[:, p0:p0 + nsf, 2:82], THIRD,
                sf[:, p0:p0 + nsf, 1:81], OP.mult, OP.add)
            up = upool.tile([128, 12, OW], BF, tag="up")
            nc.vector.scalar_tensor_tensor(
                up[:, 0:2 * npair:2, :], wt[:, 0:npair, :], THIRD,
                wt[:, 1:1 + npair, :], OP.mult, OP.add)
            nc.vector.scalar_tensor_tensor(
                up[:, 1:2 * npair:2, :], wt[:, 2:2 + npair, :], THIRD,
                wt[:, 1:1 + npair, :], OP.mult, OP.add)
            sub = npair * 80          # cols per quarter band
            rsub = npair // 2         # output rows per quarter band
            band = up[:].rearrange("k r w -> k (r w)")
            for (n0, nn) in N_TILES:
                big = stg.tile([128, 4, sub], F32, tag="big", bufs=2)
                for half_ in range(2):
                    eps_ = pp_mm.tile([nn, 2, 512], F32, tag="mm")
                    for sid2 in range(2):
                        sid = half_ * 2 + sid2
                        cols = band[:, sid * sub:(sid + 1) * sub]
                        nc.tensor.matmul(eps_[:, sid2, :sub], qT[:, n0:n0 + nn], cols,
                                         start=True, stop=True)
                    if half_ == 0:
                        nc.scalar.activation(big[:nn, 0:2, :], eps_[:, :, :sub],
                                             AF.Identity, bias=hb[:nn])
                    else:
                        nc.vector.tensor_scalar_add(big[:nn, 2:4, :], eps_[:, :, :sub],
                                                    hb[:nn])
                nc.scalar.dma_start(
                    out=y.ap()[n0:n0 + nn, 2 * p0:2 * p0 + 2 * npair, :],
                    in_=big[:nn, :, :])

    nc.compile()
    return nc


# ---------------- host side ----------------

def prep_core_inputs(inputs, b, half):
    """Build the per-core input map. half==1 is H-mirrored."""
    o = {}
    x = np.asarray(inputs["spatial_features"])[b]
    if half == 0:
        strip = x[:, 0:R, :]
    else:
        strip = x[:, 79:79 - R:-1, :]
    xp = np.zeros((2, 128, PR, PW), np.float32)
    xp[0, :, 1:44, 1:81] = strip[:128]
    xp[1, :, 1:44, 1:81] = strip[128:]
    o["xin"] = xp

    dw_w = np.asarray(inputs["dw_w"])  # [2,256,1,3,3]
    if half == 1:
        dw_w = dw_w[:, :, :, ::-1, :]  # flip dy
    diag = np.zeros((2, 9, 2, 128, 128), np.float32)
    eye = np.arange(128)
    for blk in range(2):
        for t, (dy, dx) in enumerate(TAPS):
            for ct in range(2):
                w = dw_w[blk, ct * 128:(ct + 1) * 128, 0, dy + 1, dx + 1]
                diag[blk, t, ct, eye, eye] = w
    o["dw_diag"] = diag
    dwb = np.zeros((2, 2, 128, 1), np.float32)  # [blk, ct, 128, 1]
    for blk in range(2):
        for ct in range(2):
            dwb[blk, ct, :, 0] = np.asarray(inputs["dw_b"])[blk, ct * 128:(ct + 1) * 128]
    o["dwb"] = dwb

    ln_w = np.asarray(inputs["ln_w"]); ln_b = np.asarray(inputs["ln_b"])
    pw_w = np.asarray(inputs["pw_w"]); pw_b = np.asarray(inputs["pw_b"])
    Wt = np.zeros((2, 2, 2, 128, 128), np.float32)
    colW = np.zeros((2, 2, 128, 1), np.float32)
    pwbe = np.zeros((2, 2, 128, 1), np.float32)
    for blk in range(2):
        We = pw_w[blk] * ln_w[blk][None, :]
        for kt in range(2):
            for mt in range(2):
                Wt[blk, kt, mt] = We[mt * 128:(mt + 1) * 128, kt * 128:(kt + 1) * 128].T
        for mt in range(2):
            colW[blk, mt, :, 0] = We[mt * 128:(mt + 1) * 128].sum(1)
            pwbe[blk, mt, :, 0] = (pw_b[blk] + pw_w[blk] @ ln_b[blk])[mt * 128:(mt + 1) * 128]
    o["pw_lhsT"] = Wt; o["pw_colwp"] = colW; o["pw_be"] = pwbe

    sp = np.asarray(inputs["sproj_w"])
    o["sproj_lhsT"] = np.stack([sp[:, :128].T, sp[:, 128:].T]).astype(np.float32)
    o["sproj_b"] = np.asarray(inputs["sproj_b"]).reshape(128, 1).astype(np.float32)

    qf = np.asarray(inputs["query_features"])[b]
    o["qfT"] = np.ascontiguousarray(qf.T).reshape(2, 128, NQ).astype(np.float32)
    w1 = np.asarray(inputs["mlp_w1"])
    o["w1T"] = np.stack([w1[:, k * 128:(k + 1) * 128].T for k in range(2)]).astype(np.float32)
    o["b1"] = np.asarray(inputs["mlp_b1"]).reshape(4, 128, 1).astype(np.float32)
    w2 = np.asarray(inputs["mlp_w2"])
    o["w2T"] = np.stack([w2[:, k * 128:(k + 1) * 128].T for k in range(4)]).astype(np.float32)
    o["b2"] = np.asarray(inputs["mlp_b2"]).reshape(4, 128, 1).astype(np.float32)
    w3q = 0.5625 * (np.asarray(inputs["qproj_w"]) @ np.asarray(inputs["mlp_w3"]))
    o["w3qT"] = np.stack([w3q[:, k * 128:(k + 1) * 128].T for k in range(4)]).astype(np.float32)
    o["b3q"] = (0.5625 * (np.asarray(inputs["qproj_w"]) @ np.asarray(inputs["mlp_b3"])
                + np.asarray(inputs["qproj_b"]))).reshape(128, 1).astype(np.float32)
    o["hb"] = np.full((128, 1), float(np.asarray(inputs["head_bias"])[0]), np.float32)
    o = {k: np.ascontiguousarray(
            v, dtype=(ml_dtypes.bfloat16 if k in BF16_INPUTS else np.float32))
         for k, v in o.items()}
    return o


_NC_CACHE = {}


def get_runner():
    if "runner" not in _NC_CACHE:
        _NC_CACHE["nc"] = build_program()
        _NC_CACHE["runner"] = CachedRunner(_NC_CACHE["nc"], 8)
    return _NC_CACHE["runner"]


def kernel(**inputs):
    runner = get_runner()
    in_maps = [prep_core_inputs(inputs, core // 2, core % 2) for core in range(8)]
    res = runner(in_maps)
    out = np.empty((4, NQ, 160, 160), np.float32)
    for core in range(8):
        b, half = core // 2, core % 2
        yc = res[core]["y"]
        if half == 0:
            out[b, :, 0:80, :] = yc
        else:
            out[b, :, 80:160, :] = yc[:, ::-1, :]
    return out



# revision 34
# speedup vs baseline: 1.3521x; 1.3521x over previous
"""Self-contained Trainium2 Bass kernel for nn_DFINESegTransformer.

kernel(**inputs) -> [4, 300, 160, 160] float32 mask logits.

Sharding: 8 NeuronCores = 4 batches x 2 output-row halves. Every core runs
an identical Bass/Tile program; the host mirrors (flips H) the input strip
and the depthwise taps for odd halves and un-flips their output rows.

Per-core pipeline: 2x ConvNeXt block [dwconv3x3 as 9 diagonal PE matmuls ->
channel-LN via ones-matmul stats + folded affine -> 1x1 conv (ln_w folded)
-> exact Gelu (ACT table) -> residual] -> sproj 1x1 -> separable bilinear 2x
upsample (scale factors folded into query weights) -> per-query mask einsum.
Heavy operands in bf16 (PE streams 1 col/cycle vs 4 for fp32); LN stats,
alpha/beta math, gelu input and the output stay fp32.
"""
import numpy as np
import ml_dtypes

import jax
from jax.sharding import Mesh, PartitionSpec
try:
    from jax.experimental.shard_map import shard_map
except ImportError:
    from jax.shard_map import shard_map

import concourse.bacc as bacc
import concourse.bass as bass
import concourse.tile as tile
from concourse import mybir
from concourse.bass2jax import (_bass_exec_p, install_neuronx_cc_hook,
                                partition_id_tensor)
from contextlib import ExitStack

class CachedRunner:
    def __init__(self, nc, n_cores=8):
        install_neuronx_cc_hook()
        self.n_cores = n_cores
        partition_name = nc.partition_id_tensor.name if nc.partition_id_tensor else None
        in_names, out_names, out_avals, zero_shapes = [], [], [], []
        for alloc in nc.m.functions[0].allocations:
            if not isinstance(alloc, mybir.MemoryLocationSet):
                continue
            name = alloc.memorylocations[0].name
            if alloc.kind == "ExternalInput":
                if name != partition_name:
                    in_names.append(name)
            elif alloc.kind == "ExternalOutput":
                out_names.append(name)
                shape = tuple(alloc.tensor_shape)
                dtype = mybir.dt.np(alloc.dtype)
                out_avals.append(jax.core.ShapedArray(shape, dtype))
                zero_shapes.append((shape, dtype))
        self.n_params = len(in_names)
        self.in_names = list(in_names)
        self.out_names = out_names
        self.out_avals = out_avals
        self.zero_shapes = zero_shapes
        all_in = in_names + out_names
        if partition_name is not None:
            all_in.append(partition_name)

        donate = tuple(range(self.n_params, self.n_params + len(out_names)))

        def _body(*args):
            operands = list(args)
            if partition_name is not None:
                operands.append(partition_id_tensor())
            outs = _bass_exec_p.bind(
                *operands,
                out_avals=tuple(out_avals),
                in_names=tuple(all_in),
                out_names=tuple(out_names),
                lowering_input_output_aliases=(),
                sim_require_finite=True,
                sim_require_nnan=True,
                nc=nc,
            )
            return tuple(outs)

        devices = jax.devices()[:n_cores]
        self.mesh = Mesh(np.asarray(devices), ("core",))
        in_specs = (PartitionSpec("core"),) * (self.n_params + len(out_names))
        out_specs = (PartitionSpec("core"),) * len(out_names)
        self.fn = jax.jit(
            shard_map(_body, mesh=self.mesh, in_specs=in_specs,
                      out_specs=out_specs, check_rep=False),
            donate_argnums=donate, keep_unused=True)

    def concat_inputs(self, in_maps):
        return [np.concatenate([np.asarray(m[n]) for m in in_maps], axis=0)
                for n in self.in_names]

    def zeros(self):
        return [np.zeros((self.n_cores * s[0], *s[1:]), d)
                for (s, d) in self.zero_shapes]

    def run_device(self, concat_in, zeros):
        """Returns device arrays (not transferred)."""
        outs = self.fn(*concat_in, *zeros)
        return outs

    def __call__(self, in_maps):
        outs = self.run_device(self.concat_inputs(in_maps), self.zeros())
        res = []
        for c in range(self.n_cores):
            res.append({
                n: np.asarray(outs[i]).reshape(self.n_cores, *self.out_avals[i].shape)[c]
                for i, n in enumerate(self.out_names)})
        return res


F32 = mybir.dt.float32
BF = mybir.dt.bfloat16
FP8 = mybir.dt.float8e4
AF = mybir.ActivationFunctionType
OP = mybir.AluOpType

C = 256
EPS = 1e-6
R = 43            # strip rows per core
PR, PW = 45, 82   # padded strip
MR = 41           # mask rows (sproj output rows)
SFR = 42          # sf rows incl. replicated top row
NQ = 300
OH, OW = 80, 160  # output rows/cols per core

DW_CHUNKS = [(i * 6, min(R, i * 6 + 6)) for i in range((R + 5) // 6)]
SP_CHUNKS = [(i * 6, min(MR, i * 6 + 6)) for i in range((MR + 5) // 6)]
UP_CHUNKS = [(i * 6, min(40, i * 6 + 6)) for i in range((40 + 5) // 6)]

TAPS = [(dy, dx) for dy in (-1, 0, 1) for dx in (-1, 0, 1)]
PE_TAPS = TAPS

N_TILES = [(0, 128), (128, 128), (256, 44)]
SROW = 3456  # padded stats row length (27*128)

BF16_INPUTS = {"xin", "pw_lhsT", "sproj_lhsT", "qfT",
               "w1T", "w2T", "w3qT"}
FP8_INPUTS = {"xin8", "dw_pr", "dw_sg"}
# dw taps paired for fp8 DoubleRow matmuls (2 taps per PE pass, 0.5 cyc/row);
# tap 8 runs as a plain fp8 matmul.  dw weights are scaled x64 on the host so
# they sit in fp8e4m3's normal range; LN's alpha/beta absorb the scale exactly.
DW_PAIRS = [(0, 1), (2, 3), (4, 5), (6, 7)]
DW_SINGLE = 8
# All inputs are stored partition-major on the host so every SBUF-load DMA
# moves one long contiguous run per partition (few descriptors, full DGE speed)
INPUT_SPECS = [
    ("xin", (128, 2, PR, PW)),
    ("xin8", (128, 2, PR, PW)),
    ("dw_pr", (2, 128, 2, 4, 2, 128)),
    ("dw_sg", (2, 128, 2, 128)),
    ("dwb", (128, 2, 2)),
    ("pw_lhsT", (128, 2, 2, 2, 128)),
    ("pw_colwp", (128, 2, 2)),
    ("pw_be", (128, 2, 2)),
    ("sproj_lhsT", (128, 2, 128)),
    ("sproj_b", (128, 1)),
    ("qfT", (128, 2, NQ)),
    ("w1T", (128, 2, 512)),
    ("b1", (128, 4)),
    ("w2T", (128, 4, 512)),
    ("b2", (128, 4)),
    ("w3qT", (128, 4, 128)),
    ("b3q", (128, 1)),
    ("hb", (128, 1)),
]


def build_program():
    nc = bacc.Bacc("TRN2", target_bir_lowering=False, debug=False, num_devices=8)

    di = {}
    for name, shape in INPUT_SPECS:
        dt = BF if name in BF16_INPUTS else (FP8 if name in FP8_INPUTS else F32)
        di[name] = nc.dram_tensor(name, list(shape), dt, kind="ExternalInput")
    y = nc.dram_tensor("y", [NQ, OH, OW], F32, kind="ExternalOutput")
    scratch = nc.dram_tensor("scratch", [2, SROW], BF)

    with tile.TileContext(nc) as tc, ExitStack() as ctx:
        consts = ctx.enter_context(tc.tile_pool(name="consts", bufs=1))
        states = ctx.enter_context(tc.tile_pool(name="states", bufs=2))
        zpool = ctx.enter_context(tc.tile_pool(name="zpool", bufs=2))
        sfpool = ctx.enter_context(tc.tile_pool(name="sfpool", bufs=1))
        small = ctx.enter_context(tc.tile_pool(name="small", bufs=10))
        upool = ctx.enter_context(tc.tile_pool(name="upool", bufs=2))
        stg = ctx.enter_context(tc.tile_pool(name="stg", bufs=4))
        qpool = ctx.enter_context(tc.tile_pool(name="qpool", bufs=1))
        dwpool = ctx.enter_context(tc.tile_pool(name="dwpool", bufs=1))
        rowp = ctx.enter_context(tc.tile_pool(name="rowp", bufs=1))
        bcp = ctx.enter_context(tc.tile_pool(name="bcp", bufs=1))

        pp_z = ctx.enter_context(tc.tile_pool(name="pp_z", bufs=2, space="PSUM"))
        pp_mm = ctx.enter_context(tc.tile_pool(name="pp_mm", bufs=2, space="PSUM"))
        pp_eps = ctx.enter_context(tc.tile_pool(name="pp_eps", bufs=4, space="PSUM"))

        # ---- weights/constants -> SBUF ----
        dwb = consts.tile([128, 2, 2], F32, tag="dwb")
        pw_lhsT = consts.tile([128, 2, 2, 2, 128], BF, tag="pw_lhsT")
        pw_colwp = consts.tile([128, 2, 2], F32, tag="pw_colwp")
        pw_be = consts.tile([128, 2, 2], F32, tag="pw_be")
        sproj_lhsT = consts.tile([128, 2, 128], BF, tag="sproj_lhsT")
        sproj_b = consts.tile([128, 1], F32, tag="sproj_b")
        qfT = consts.tile([128, 2, NQ], BF, tag="qfT")
        w1T = consts.tile([128, 2, 512], BF, tag="w1T")
        b1 = consts.tile([128, 4], F32, tag="b1")
        w2T = consts.tile([128, 4, 512], BF, tag="w2T")
        b2 = consts.tile([128, 4], F32, tag="b2")
        w3qT = consts.tile([128, 4, 128], BF, tag="w3qT")
        b3q = consts.tile([128, 1], F32, tag="b3q")
        hb = consts.tile([128, 1], F32, tag="hb")
        ones_col = consts.tile([128, 1], BF, tag="ones_col")
        eps_t = consts.tile([128, 1], F32, tag="eps_t")
        ones_row = consts.tile([1, 128], F32, tag="ones_row")

        A = lambda n: di[n].ap()
        nc.sync.dma_start(out=dwb[:], in_=A("dwb"))
        nc.sync.dma_start(out=pw_lhsT[:], in_=A("pw_lhsT"))
        nc.sync.dma_start(out=pw_colwp[:], in_=A("pw_colwp"))
        nc.sync.dma_start(out=pw_be[:], in_=A("pw_be"))
        nc.sync.dma_start(out=sproj_lhsT[:], in_=A("sproj_lhsT"))
        nc.sync.dma_start(out=sproj_b[:], in_=A("sproj_b"))
        nc.sync.dma_start(out=qfT[:], in_=A("qfT"))
        nc.sync.dma_start(out=w1T[:], in_=A("w1T"))
        nc.sync.dma_start(out=b1[:], in_=A("b1"))
        nc.sync.dma_start(out=w2T[:], in_=A("w2T"))
        nc.sync.dma_start(out=b2[:], in_=A("b2"))
        nc.sync.dma_start(out=w3qT[:], in_=A("w3qT"))
        nc.sync.dma_start(out=b3q[:], in_=A("b3q"))
        nc.sync.dma_start(out=hb[:], in_=A("hb"))
        nc.vector.memset(ones_col[:], 1.0 / C)
        nc.vector.memset(eps_t[:], EPS)

        nc.vector.memset(ones_row[:], 1.0)

        x_state = states.tile([128, 2, PR, PW], BF, tag="state")
        xin_r = A("xin")
        x8_state = states.tile([128, 2, PR, PW], FP8, tag="state8")
        nc.sync.dma_start(out=x8_state[:, :, 0:15, :], in_=A("xin8")[:, :, 0:15, :])
        nc.scalar.dma_start(out=x8_state[:, :, 15:27, :], in_=A("xin8")[:, :, 15:27, :])
        nc.sync.dma_start(out=x8_state[:, :, 27:PR, :], in_=A("xin8")[:, :, 27:PR, :])
        nc.scalar.dma_start(out=x_state[:, :, 0:14, :], in_=xin_r[:, :, 0:14, :])
        nc.sync.dma_start(out=x_state[:, :, 14:28, :], in_=xin_r[:, :, 14:28, :])
        nc.scalar.dma_start(out=x_state[:, :, 28:PR, :], in_=xin_r[:, :, 28:PR, :])

        # ---- query MLP ----
        q1 = qpool.tile([128, 4, NQ], BF, tag="q1")
        for mt in range(4):
            ps = pp_mm.tile([128, NQ], F32, tag="mm")
            for kt in range(2):
                nc.tensor.matmul(ps[:], w1T[:, kt, mt * 128:(mt + 1) * 128],
                                 qfT[:, kt, :], start=(kt == 0), stop=(kt == 1))
            nc.scalar.activation(q1[:, mt, :], ps[:], AF.Relu, bias=b1[:, mt:mt + 1])
        q2 = qpool.tile([128, 4, NQ], BF, tag="q2")
        for mt in range(4):
            ps = pp_mm.tile([128, NQ], F32, tag="mm")
            for kt in range(4):
                nc.tensor.matmul(ps[:], w2T[:, kt, mt * 128:(mt + 1) * 128],
                                 q1[:, kt, :], start=(kt == 0), stop=(kt == 3))
            nc.scalar.activation(q2[:, mt, :], ps[:], AF.Relu, bias=b2[:, mt:mt + 1])
        qT = qpool.tile([128, NQ], BF, tag="qT")
        ps = pp_mm.tile([128, NQ], F32, tag="mm")
        for kt in range(4):
            nc.tensor.matmul(ps[:], w3qT[:, kt, :], q2[:, kt, :],
                             start=(kt == 0), stop=(kt == 3))
        nc.scalar.activation(qT[:], ps[:], AF.Identity, bias=b3q[:])

        # ---- conv blocks ----
        # Stats/alpha-beta are split into two halves so pass B of half 0
        # overlaps pass A of half 1 on the PE.  Halves cover position ranges
        # [0, 1920) and [1920, 3456) -- both multiples of 128 wide, so the
        # pack/unpack transpose DMAs stay aligned.
        H_CH = [DW_CHUNKS[:4], DW_CHUNKS[4:]]          # chunks per half
        H_OFF = [0, 1920]                              # position offset
        H_W = [15, 12]                                 # packed cols (128*w)

        def conv_block(blk, xst, x8, pad_out, schedule=None):
            # warm the sqrt act table off the alpha/beta critical chain
            warm = small.tile([1, 1], F32, tag="warm", bufs=2)
            nc.scalar.activation(warm[:], eps_t[0:1, 0:1], AF.Sqrt)
            dwp = dwpool.tile([128, 2, 4, 2, 128], FP8, tag="dwp")
            nc.gpsimd.dma_start(out=dwp[:], in_=A("dw_pr")[blk])
            dws = dwpool.tile([128, 2, 128], FP8, tag="dws")
            nc.gpsimd.dma_start(out=dws[:], in_=A("dw_sg")[blk])
            z = zpool.tile([128, 2, R, 80], BF, tag="z")
            um_rows = rowp.tile([33, SROW], F32, tag="um_rows")
            pad0 = 3440 if blk == 0 else 3360
            nc.vector.memset(um_rows[0:1, pad0:], 0.0)
            nc.vector.memset(um_rows[32:33, pad0:], 1.0)
            a_bc = bcp.tile([128, SROW], BF, tag="a_bc")
            b_bc = bcp.tile([128, SROW], BF, tag="b_bc")
            out = states.tile([128, 2, PR, PW], BF, tag="state")
            out8 = states.tile([128, 2, PR, PW], FP8, tag="state8", name="out8") if blk == 0 else None
            if pad_out:
                nc.gpsimd.memset(out[:, :, 0, :], 0.0)
                nc.gpsimd.memset(out[:, :, PR - 1, :], 0.0)
                nc.gpsimd.memset(out[:, :, :, 0], 0.0)
                nc.gpsimd.memset(out[:, :, :, PW - 1], 0.0)
            if out8 is not None:
                nc.gpsimd.memset(out8[:, :, 0, :], 0.0)
                nc.gpsimd.memset(out8[:, :, PR - 1, :], 0.0)
                nc.gpsimd.memset(out8[:, :, :, 0], 0.0)
                nc.gpsimd.memset(out8[:, :, :, PW - 1], 0.0)

            def pass_a(chunks):
                # dw conv + stats; tap-outer over 2-chunk superchunks
                for sc0 in range(0, len(chunks), 2):
                    sch = chunks[sc0:sc0 + 2]
                    for ct in range(2):
                        zpss = []
                        for ci, (r0, r1) in enumerate(sch):
                            zps_t = pp_z.tile([128, r1 - r0, 80], F32, tag="zs", name="zps")
                            zpss.append(zps_t)
                        for i, (ta, tb) in enumerate(DW_PAIRS):
                            dya, dxa = TAPS[ta]
                            dyb, dxb = TAPS[tb]
                            delta = (dyb - dya) * PW + (dxb - dxa)
                            for ci, (r0, r1) in enumerate(sch):
                                base = x8[:, ct, 1 + r0 + dya:1 + r1 + dya,
                                          1 + dxa:81 + dxa]
                                pair_ap = bass.AP(
                                    tensor=base.tensor, offset=base.offset,
                                    ap=[base.ap[0], [delta, 2],
                                        base.ap[1], base.ap[2]])
                                nc.tensor.matmul(
                                    zpss[ci][:], dwp[:, ct, i, :, :], pair_ap,
                                    start=(i == 0), stop=False,
                                    perf_mode=mybir.MatmulPerfMode.DoubleRow)
                        dys, dxs = TAPS[DW_SINGLE]
                        for ci, (r0, r1) in enumerate(sch):
                            nc.tensor.matmul(
                                zpss[ci][:], dws[:, ct, :],
                                x8[:, ct, 1 + r0 + dys:1 + r1 + dys,
                                   1 + dxs:81 + dxs],
                                start=False, stop=True)
                        for ci, (r0, r1) in enumerate(sch):
                            zc = z[:, ct, r0:r1, :]
                            nc.scalar.activation(zc, zpss[ci][:], AF.Identity,
                                                 bias=dwb[:, blk, ct:ct + 1])
                    # stats for this superchunk (both ctiles of z ready)
                    for (r0, r1) in sch:
                        nr = r1 - r0
                        stps = pp_z.tile([33, nr * 80], F32, tag="zs", name="stat")
                        for ct in range(2):
                            zc = z[:, ct, r0:r1, :]
                            zsq = small.tile([128, nr, 80], BF, tag="tmp")
                            nc.vector.tensor_tensor(zsq[:], zc, zc, OP.mult)
                            nc.tensor.matmul(stps[0:1, :], ones_col[:], zc,
                                             start=(ct == 0), stop=(ct == 1),
                                             skip_group_check=True)
                            nc.tensor.matmul(stps[32:33, :], ones_col[:], zsq[:],
                                             start=(ct == 0), stop=(ct == 1),
                                             skip_group_check=True)
                        nc.scalar.copy(um_rows[0:1, r0 * 80:r1 * 80], stps[0:1, :])
                        nc.vector.tensor_copy(um_rows[32:33, r0 * 80:r1 * 80],
                                              stps[32:33, :])

            def alpha_beta(h):
                # stps rows already hold u/C and m2/C (1/C folded into the
                # stats lhsT); alpha = (var+eps)^-0.5 fused on gpsimd pow so
                # the Act engine (and its sqrt-table load) stays off the chain
                off, w = H_OFF[h], H_W[h]
                t_pk = small.tile([128, 15], F32, tag="pk")
                m2_pk = small.tile([128, 15], F32, tag="pk")
                nc.sync.dma_start(out=t_pk[:, :w], in_=um_rows[0:1, off:off + 128 * w])
                nc.scalar.dma_start(out=m2_pk[:, :w], in_=um_rows[32:33, off:off + 128 * w])
                sq = small.tile([128, 15], F32, tag="pk")
                nc.vector.tensor_tensor(sq[:, :w], t_pk[:, :w], t_pk[:, :w], OP.mult)
                d_pk = small.tile([128, 15], F32, tag="pk")
                nc.vector.tensor_tensor(d_pk[:, :w], m2_pk[:, :w], sq[:, :w],
                                        OP.subtract)
                sd = small.tile([128, 15], F32, tag="pk")
                nc.scalar.activation(sd[:, :w], d_pk[:, :w], AF.Sqrt, bias=eps_t[:])
                a_pk = small.tile([128, 15], BF, tag="pk")
                b_pk = small.tile([128, 15], BF, tag="pk")
                with nc.allow_low_precision(reason="alpha/beta rows cast to bf16"):
                    nc.vector.reciprocal(a_pk[:, :w], sd[:, :w])
                    nc.vector.tensor_scalar_mul(b_pk[:, :w], t_pk[:, :w], -1.0)
                nc.sync.dma_start(out=scratch.ap()[0:1, off:off + 128 * w],
                                  in_=a_pk[:, :w])
                nc.scalar.dma_start(out=scratch.ap()[1:2, off:off + 128 * w],
                                    in_=b_pk[:, :w])
                seg = 128 * w
                src_a = bass.AP(tensor=scratch, offset=off,
                                ap=[[0, 128], [1, seg]])
                src_b = bass.AP(tensor=scratch, offset=SROW + off,
                                ap=[[0, 128], [1, seg]])
                nc.scalar.dma_start(out=a_bc[:, off:off + seg], in_=src_a)
                nc.sync.dma_start(out=b_bc[:, off:off + seg], in_=src_b)

            def pass_b(chunks):
                for (r0, r1) in chunks:
                    nr = r1 - r0
                    for mt in range(2):
                        pwps = pp_mm.tile([128, nr, 80], F32, tag="mm")
                        for kt in range(2):
                            nc.tensor.matmul(pwps[:], pw_lhsT[:, blk, kt, mt, :],
                                             z[:, kt, r0:r1, :],
                                             start=(kt == 0), stop=(kt == 1))
                        t1 = small.tile([128, nr, 80], F32, tag="tmp")
                        nc.vector.scalar_tensor_tensor(
                            t1[:], b_bc[:, r0 * 80:r1 * 80].rearrange("k (r w) -> k r w", w=80),
                            pw_colwp[:, blk, mt:mt + 1], pwps[:], OP.mult, OP.add)
                        g = small.tile([128, nr, 80], F32, tag="tmp")
                        nc.gpsimd.tensor_tensor(
                            g[:], t1[:],
                            a_bc[:, r0 * 80:r1 * 80].rearrange("k (r w) -> k r w", w=80),
                            OP.mult)
                        gel = small.tile([128, nr, 80], BF, tag="tmp")
                        nc.scalar.activation(gel[:], g[:], AF.Gelu,
                                             bias=pw_be[:, blk, mt:mt + 1])
                        reng = nc.vector if mt == 0 else nc.gpsimd
                        reng.tensor_tensor(
                            out[:, mt, 1 + r0:1 + r1, 1:81], gel[:],
                            xst[:, mt, 1 + r0:1 + r1, 1:81], OP.add)
                        if out8 is not None:
                            with nc.allow_low_precision(reason="fp8 dw input"):
                                if mt == 0:
                                    nc.vector.tensor_copy(
                                        out8[:, mt, 1 + r0:1 + r1, 1:81],
                                        out[:, mt, 1 + r0:1 + r1, 1:81])
                                else:
                                    nc.scalar.copy(
                                        out8[:, mt, 1 + r0:1 + r1, 1:81],
                                        out[:, mt, 1 + r0:1 + r1, 1:81])

            h2 = H_CH[1] if blk == 0 else H_CH[1][:-1]   # blk1: row 42 unused
            pass_a(H_CH[0])
            alpha_beta(0)
            pass_a(h2)
            alpha_beta(1)
            for ci, ch in enumerate(H_CH[0] + h2):
                pass_b([ch])
                if schedule and ci in schedule:
                    schedule[ci](out)
            return out, out8

        # ---- sproj -> sf, upsample + einsum + out (interleaved per chunk) ----
        # sf/sf3 stored as [128, SFR, PW]; wt/up use [rows, 2, 80] even|odd
        # layout (contiguous writes -> DVE 2x bf16); the einsum rhs AP
        # re-interleaves to y order for free on the PE.
        sf = sfpool.tile([128, SFR, PW], BF, tag="sf")
        sf3 = sfpool.tile([128, SFR, PW], BF, tag="sf3")
        wtf = sfpool.tile([128, SFR, 2, 80], BF, tag="wtf")
        wt3f = sfpool.tile([128, SFR, 2, 80], BF, tag="wt3f")
        THIRD = 1.0 / 3.0

        def lerp_rows(rr0, rr1):
            nr = rr1 - rr0
            sfv = sf[:, rr0:rr1, 1:81]
            sf_ctr = bass.AP(tensor=sfv.tensor, offset=sfv.offset,
                             ap=[sfv.ap[0], sfv.ap[1], [0, 2], [1, 80]])
            s3v = sf3[:, rr0:rr1, 0:80]
            sf3_eo = bass.AP(tensor=s3v.tensor, offset=s3v.offset,
                             ap=[s3v.ap[0], s3v.ap[1], [2, 2], [1, 80]])
            nc.vector.tensor_tensor(wtf[:, rr0:rr1, :, :], sf_ctr, sf3_eo, OP.add)
            nc.vector.tensor_scalar_mul(wt3f[:, rr0:rr1, :, :],
                                        wtf[:, rr0:rr1, :, :], THIRD)

        def sproj_chunk(ci, b2st):
            r0, r1 = SP_CHUNKS[ci]
            nr = r1 - r0
            sps = pp_mm.tile([128, nr, 80], F32, tag="mm")
            for kt in range(2):
                nc.tensor.matmul(sps[:], sproj_lhsT[:, kt, :],
                                 b2st[:, kt, 1 + r0:1 + r1, 1:81],
                                 start=(kt == 0), stop=(kt == 1))
            nc.scalar.activation(sf[:, 1 + r0:1 + r1, 1:81], sps[:], AF.Identity,
                                 bias=sproj_b[:])
            nc.vector.tensor_copy(sf[:, 1 + r0:1 + r1, 0], sf[:, 1 + r0:1 + r1, 1])
            nc.vector.tensor_copy(sf[:, 1 + r0:1 + r1, 81], sf[:, 1 + r0:1 + r1, 80])
            nc.vector.tensor_scalar_mul(sf3[:, 1 + r0:1 + r1, :],
                                        sf[:, 1 + r0:1 + r1, :], THIRD)
            if ci == 0:
                nc.vector.tensor_copy(sf[:, 0, :], sf[:, 1, :])
                nc.vector.tensor_scalar_mul(sf3[:, 0:1, :], sf[:, 0:1, :], THIRD)
                lerp_rows(0, 1 + r1)
            else:
                lerp_rows(1 + r0, 1 + r1)

        def up_chunk(uc, eo_cycle=[0]):
            p0, p1 = UP_CHUNKS[uc]
            npair = p1 - p0
            nsf = npair + 2
            up = upool.tile([128, 12, 2, 80], BF, tag="up", bufs=2)
            wv = wtf[:, p0 + 1:p0 + 1 + npair, :, :]
            wt_ctr = bass.AP(tensor=wv.tensor, offset=wv.offset,
                             ap=[wv.ap[0], wv.ap[1], [0, 2], [1, 160]])
            w3v = wt3f[:, p0:p0 + npair, :, :]
            wt3_d = bass.AP(tensor=w3v.tensor, offset=w3v.offset,
                            ap=[w3v.ap[0], w3v.ap[1], [320, 2], [1, 160]])
            uv = up[:, 0:2 * npair, :, :]
            up_d = bass.AP(tensor=uv.tensor, offset=uv.offset,
                           ap=[uv.ap[0], [320, npair], [160, 2], [1, 160]])
            nc.vector.tensor_tensor(up_d, wt_ctr, wt3_d, OP.add)
            sub = npair * 80          # cols per quarter band
            rps = npair // 2          # up rows per quarter band
            for (n0, nn) in N_TILES:
                big = stg.tile([128, 4, sub], F32, tag="big", bufs=3)
                for sid in range(4):
                    rr = rps * sid
                    cols = up[:, rr:rr + rps, :, :].rearrange("k r e j -> k r j e")
                    eps1 = pp_eps.tile([128, 480], F32, tag="eps")
                    nc.tensor.matmul(eps1[:nn, :sub], qT[:, n0:n0 + nn],
                                     cols, start=True, stop=True)
                    eng = (nc.scalar, nc.vector)[eo_cycle[0] % 2]
                    eo_cycle[0] += 1
                    if eng is nc.scalar:
                        nc.scalar.activation(big[:nn, sid, :], eps1[:nn, :sub],
                                             AF.Identity, bias=hb[:nn])
                    else:
                        eng.tensor_scalar(big[:nn, sid, :], eps1[:nn, :sub],
                                          1.0, hb[:nn], OP.mult, OP.add)
                deng = nc.scalar if (uc % 2 == 0) else nc.sync
                deng.dma_start(
                    out=y.ap()[n0:n0 + nn, 2 * p0:2 * p0 + 2 * npair, :],
                    in_=big[:nn, :, :])

        # interleave: up_chunk(k) needs sf rows <= 6k+7 -> sproj chunks <= k+1;
        # sproj chunk k needs conv2 pass-B chunk k.  The schedule fires tail
        # work right after the B chunk that unblocks it.
        b1st, b1st8 = conv_block(0, x_state, x8_state, True)
        sched = {
            1: lambda o: (sproj_chunk(0, o), sproj_chunk(1, o), up_chunk(0)),
            3: lambda o: (sproj_chunk(2, o), sproj_chunk(3, o),
                          up_chunk(1), up_chunk(2)),
            4: lambda o: (sproj_chunk(4, o), up_chunk(3)),
            5: lambda o: (sproj_chunk(5, o), up_chunk(4)),
            6: lambda o: (sproj_chunk(6, o), up_chunk(5), up_chunk(6)),
        }
        b2st, _ = conv_block(1, b1st, b1st8, False, schedule=sched)

    nc.compile()
    return nc


# ---------------- host side ----------------

def prep_core_inputs(inputs, b, half):
    """Build the per-core input map. half==1 is H-mirrored."""
    o = {}
    x = np.asarray(inputs["spatial_features"])[b]
    if half == 0:
        strip = x[:, 0:R, :]
    else:
        strip = x[:, 79:79 - R:-1, :]
    xp = np.zeros((128, 2, PR, PW), np.float32)
    xp[:, 0, 1:44, 1:81] = strip[:128]
    xp[:, 1, 1:44, 1:81] = strip[128:]
    o["xin"] = xp
    o["xin8"] = xp

    dw_w = np.asarray(inputs["dw_w"])  # [2,256,1,3,3]
    if half == 1:
        dw_w = dw_w[:, :, :, ::-1, :]  # flip dy
    eye = np.arange(128)
    dw_pr = np.zeros((2, 128, 2, 4, 2, 128), np.float32)  # [blk,k,ct,pi,j,m]
    dw_sg = np.zeros((2, 128, 2, 128), np.float32)
    for blk in range(2):
        for ct in range(2):
            for pi, pr in enumerate(DW_PAIRS):
                for j, t in enumerate(pr):
                    dy, dx = TAPS[t]
                    w = dw_w[blk, ct * 128:(ct + 1) * 128, 0, dy + 1, dx + 1]
                    dw_pr[blk, eye, ct, pi, j, eye] = 64.0 * w
            dy, dx = TAPS[DW_SINGLE]
            w = dw_w[blk, ct * 128:(ct + 1) * 128, 0, dy + 1, dx + 1]
            dw_sg[blk, eye, ct, eye] = 64.0 * w
    o["dw_pr"] = dw_pr
    o["dw_sg"] = dw_sg
    dwb = np.zeros((128, 2, 2), np.float32)        # [k, blk, ct]
    for blk in range(2):
        for ct in range(2):
            dwb[:, blk, ct] = 64.0 * np.asarray(inputs["dw_b"])[blk, ct * 128:(ct + 1) * 128]
    o["dwb"] = dwb

    ln_w = np.asarray(inputs["ln_w"]); ln_b = np.asarray(inputs["ln_b"])
    pw_w = np.asarray(inputs["pw_w"]); pw_b = np.asarray(inputs["pw_b"])
    Wt = np.zeros((2, 2, 2, 128, 128), np.float32)
    colW = np.zeros((128, 2, 2), np.float32)       # [k, blk, mt]
    pwbe = np.zeros((128, 2, 2), np.float32)
    for blk in range(2):
        We = pw_w[blk] * ln_w[blk][None, :]
        for kt in range(2):
            for mt in range(2):
                Wt[blk, kt, mt] = We[mt * 128:(mt + 1) * 128, kt * 128:(kt + 1) * 128].T
        for mt in range(2):
            colW[:, blk, mt] = We[mt * 128:(mt + 1) * 128].sum(1)
            pwbe[:, blk, mt] = (pw_b[blk] + pw_w[blk] @ ln_b[blk])[mt * 128:(mt + 1) * 128]
    o["pw_lhsT"] = Wt.transpose(3, 0, 1, 2, 4)     # [k, blk, kt, mt, m]
    o["pw_colwp"] = colW; o["pw_be"] = pwbe

    sp = np.asarray(inputs["sproj_w"])
    o["sproj_lhsT"] = np.stack([sp[:, :128].T, sp[:, 128:].T]).transpose(1, 0, 2).astype(np.float32)
    o["sproj_b"] = np.asarray(inputs["sproj_b"]).reshape(128, 1).astype(np.float32)

    qf = np.asarray(inputs["query_features"])[b]
    o["qfT"] = np.ascontiguousarray(qf.T).reshape(2, 128, NQ).transpose(1, 0, 2).astype(np.float32)
    w1 = np.asarray(inputs["mlp_w1"])
    o["w1T"] = np.stack([w1[:, k * 128:(k + 1) * 128].T for k in range(2)]).transpose(1, 0, 2).astype(np.float32)
    o["b1"] = np.asarray(inputs["mlp_b1"]).reshape(4, 128).T.astype(np.float32)
    w2 = np.asarray(inputs["mlp_w2"])
    o["w2T"] = np.stack([w2[:, k * 128:(k + 1) * 128].T for k in range(4)]).transpose(1, 0, 2).astype(np.float32)
    o["b2"] = np.asarray(inputs["mlp_b2"]).reshape(4, 128).T.astype(np.float32)
    w3q = 0.5625 * (np.asarray(inputs["qproj_w"]) @ np.asarray(inputs["mlp_w3"]))
    o["w3qT"] = np.stack([w3q[:, k * 128:(k + 1) * 128].T for k in range(4)]).transpose(1, 0, 2).astype(np.float32)
    o["b3q"] = (0.5625 * (np.asarray(inputs["qproj_w"]) @ np.asarray(inputs["mlp_b3"])
                + np.asarray(inputs["qproj_b"]))).reshape(128, 1).astype(np.float32)
    o["hb"] = np.full((128, 1), float(np.asarray(inputs["head_bias"])[0]), np.float32)
    def _dt(k):
        if k in BF16_INPUTS:
            return ml_dtypes.bfloat16
        if k in FP8_INPUTS:
            return ml_dtypes.float8_e4m3
        return np.float32
    o = {k: np.ascontiguousarray(v, dtype=_dt(k)) for k, v in o.items()}
    return o


_NC_CACHE = {}


def get_runner():
    if "runner" not in _NC_CACHE:
        _NC_CACHE["nc"] = build_program()
        _NC_CACHE["runner"] = CachedRunner(_NC_CACHE["nc"], 8)
    return _NC_CACHE["runner"]


def kernel(**inputs):
    runner = get_runner()
    in_maps = [prep_core_inputs(inputs, core // 2, core % 2) for core in range(8)]
    res = runner(in_maps)
    out = np.empty((4, NQ, 160, 160), np.float32)
    for core in range(8):
        b, half = core // 2, core % 2
        yc = res[core]["y"]
        if half == 0:
            out[b, :, 0:80, :] = yc
        else:
            out[b, :, 80:160, :] = yc[:, ::-1, :]
    return out

